# revision 9
# baseline (speedup 1.0000x reference)
"""Bahdanau-style attention kernel for Trainium2, data-parallel over batch B on 8 NeuronCores.

Reference computation (per batch b):
  attn1[p,a] = sum_c keys[p,b,c] * Wa_w[a,c] + Wa_b[a]
  attn2[a]   = sum_h queries[0,b,h] * Ua_w[a,h] + Ua_b[a]
  scores[p]  = sum_a tanh(attn1[p,a] + attn2[a]) * va_w[0,a] + va_b[0]
  weights    = softmax(scores over p)
  context[c] = sum_p weights[p] * keys[p,b,c]

Scores are tiny (|s| < ~4), so softmax is computed unnormalized: exp(s) is
accumulated into Z and context_unnorm in a single pass over keys, then
normalized at the end.

The PE contracts over the partition axis, so attn1 (contraction over c) needs
keys with c on partitions while the context accumulation (contraction over p)
needs p on partitions.  Both layouts are prepared host-side in bf16 — the
device reads 2 MB per 128-row tile (same bytes as the original f32 keys read
once) and does no on-device transposes or casts.
"""

import sys

sys.path.insert(0, "/opt/trn_rl_repo")

import ml_dtypes
import numpy as np

from concourse import bacc, bass, masks, mybir, tile
from concourse import bass_utils

P, B, C, H, A = 4096, 64, 512, 512, 256
NCORES = 8
BL = B // NCORES  # 8 batches per core
PT = 128  # rows per p-tile
NT = P // PT  # 32 p-tiles
NC_CH = C // 128  # 4 contraction chunks of 128
NH_CH = H // 128
F32 = mybir.dt.float32
BF16 = mybir.dt.bfloat16
AF = mybir.ActivationFunctionType
ALU = mybir.AluOpType


def build_nc():
    nc = bacc.Bacc("TRN2", target_bir_lowering=False, debug=False)

    # host-prepared bf16 copies of keys in both layouts
    # keysN[t, p, b, c]  = keys[t*128+p, b, c]          (p on partitions)
    # keysT[t, cc, b, mc, p] = keys[t*128+p, b, mc*128+cc]  (c-chunk on partitions)
    keysN_d = nc.dram_tensor("keysN", [NT, PT, BL, C], BF16, kind="ExternalInput")
    keysT_d = nc.dram_tensor(
        "keysT", [NT, 128, BL, NC_CH, PT], BF16, kind="ExternalInput"
    )
    q_d = nc.dram_tensor("queries", [1, BL, H], F32, kind="ExternalInput")
    waw_d = nc.dram_tensor("Wa_w", [A, C], F32, kind="ExternalInput")
    wab_d = nc.dram_tensor("Wa_b", [A], F32, kind="ExternalInput")
    uaw_d = nc.dram_tensor("Ua_w", [A, H], F32, kind="ExternalInput")
    uab_d = nc.dram_tensor("Ua_b", [A], F32, kind="ExternalInput")
    vaw_d = nc.dram_tensor("va_w", [1, A], F32, kind="ExternalInput")
    vab_d = nc.dram_tensor("va_b", [1], F32, kind="ExternalInput")
    ctx_d = nc.dram_tensor("ctx_out", [BL, C], F32, kind="ExternalOutput")
    w_d = nc.dram_tensor("w_out", [P, BL], F32, kind="ExternalOutput")

    with tile.TileContext(nc) as tc:
        with (
            tc.tile_pool(name="consts", bufs=1) as consts,
            tc.tile_pool(name="setup", bufs=2) as setup,
            tc.tile_pool(name="persist", bufs=1) as persist,
            tc.tile_pool(name="keysN", bufs=3) as keysN_pool,
            tc.tile_pool(name="keysT", bufs=3) as keysT_pool,
            tc.tile_pool(name="tanh", bufs=3) as tanh_pool,
            tc.tile_pool(name="junk", bufs=2) as junk_pool,
            tc.tile_pool(name="small", bufs=3) as small_pool,
            tc.tile_pool(name="psum_tr", bufs=1, space="PSUM") as psum_tr_pool,
            tc.tile_pool(name="psum_mm1", bufs=2, space="PSUM") as psum_mm1_pool,
            tc.tile_pool(name="psum_acc", bufs=1, space="PSUM") as psum_acc_pool,
        ):
            # ---------------- constants ----------------
            ident_bf = consts.tile([128, 128], BF16)
            masks.make_identity(nc, ident_bf[:])
            ones_col_bf = consts.tile([128, 1], BF16)
            nc.gpsimd.memset(ones_col_bf[:], 1.0)
            # padded-ones stationary for the attn2 add: row 0 = 1, rows 1.. = 0
            a2pad = consts.tile([128, PT], BF16)
            nc.gpsimd.memset(a2pad[:], 0.0)
            nc.gpsimd.memset(a2pad[0:1, :], 1.0)

            # va broadcast to all partitions (for the scores reduction)
            va_f32 = setup.tile([1, A], F32)
            nc.sync.dma_start(va_f32[:], vaw_d[:])
            va_bf_row = setup.tile([1, A], BF16)
            nc.scalar.copy(va_bf_row[:], va_f32[:])
            va_bc = consts.tile([128, A], BF16)
            nc.gpsimd.partition_broadcast(va_bc[:], va_bf_row[:])

            # va_b broadcast (bias for exp)
            vab_sb = setup.tile([1, 1], F32)
            nc.sync.dma_start(vab_sb[:], vab_d[:])
            vab_bc = consts.tile([128, 1], F32)
            nc.gpsimd.partition_broadcast(vab_bc[:], vab_sb[:])

            # ---------------- transpose the small weights ----------------
            def load_transposed(w_dram, n_k, tag):
                w_f32 = setup.tile([128, 2, n_k * 128], F32, tag="wf32")
                nc.sync.dma_start(
                    w_f32[:], w_dram.ap().rearrange("(o p) c -> p o c", p=128)
                )
                w_bf = setup.tile([128, 2, n_k * 128], BF16, tag="wbf")
                nc.scalar.copy(w_bf[:], w_f32[:])
                wT = consts.tile([128, n_k, A], BF16, tag=tag)
                for mc in range(n_k):
                    ps = psum_tr_pool.tile([128, 2, 128], BF16, tag="ktr")
                    for ac in range(2):
                        nc.tensor.transpose(
                            ps[:, ac, :],
                            w_bf[:, ac, mc * 128 : (mc + 1) * 128],
                            ident_bf[:],
                        )
                    nc.scalar.copy(wT[:, mc, :], ps[:])
                return wT

            waT = load_transposed(waw_d, NC_CH, "waT")
            uaT = load_transposed(uaw_d, NH_CH, "uaT")

            # queries -> qT [128(hh), NH_CH, BL] bf16
            q_f32 = setup.tile([BL, H], F32)
            nc.sync.dma_start(q_f32[:], q_d.ap().rearrange("q b h -> (q b) h"))
            q_bf = setup.tile([BL, H], BF16)
            nc.scalar.copy(q_bf[:], q_f32[:])
            qT = setup.tile([128, NH_CH, BL], BF16)
            for hc in range(NH_CH):
                ps = psum_tr_pool.tile([128, BL], BF16, tag="ktr")
                nc.tensor.transpose(
                    ps[:], q_bf[:, hc * 128 : (hc + 1) * 128], ident_bf[0:BL, 0:BL]
                )
                nc.scalar.copy(qT[:, hc, :], ps[:])

            # bias_sum = Wa_b + Ua_b  [1, A]
            wab_sb = setup.tile([1, A], F32, tag="bias")
            nc.sync.dma_start(wab_sb[:], wab_d.ap())
            uab_sb = setup.tile([1, A], F32, tag="bias")
            nc.sync.dma_start(uab_sb[:], uab_d.ap())
            bias_bf = setup.tile([1, A], BF16, tag="biasbf")
            nc.vector.tensor_add(bias_bf[:], wab_sb[:], uab_sb[:])

            # attn2[b, a] = q[b] @ Ua^T + Ua_b + Wa_b   -> [BL, A] psum
            ones_row_bf = setup.tile([1, 128], BF16, tag="onesrow")
            nc.gpsimd.memset(ones_row_bf[:], 1.0)
            a2_ps = psum_mm1_pool.tile([BL, A], F32, tag="mm1")
            for hc in range(NH_CH):
                nc.tensor.matmul(
                    a2_ps[:],
                    qT[:, hc, :],
                    uaT[:, hc, :],
                    start=(hc == 0),
                    stop=False,
                )
            nc.tensor.matmul(
                a2_ps[:], ones_row_bf[0:1, 0:BL], bias_bf[:], start=False, stop=True
            )
            a2_sb = setup.tile([BL, A], F32)
            nc.scalar.copy(a2_sb[:], a2_ps[:])
            # move the 8 rows to partition 0, then bf16, rows 1..127 zero
            a2_flat = setup.tile([1, BL, A], F32)
            for b in range(BL):
                nc.sync.dma_start(a2_flat[0:1, b, :], a2_sb[b : b + 1, :])
            # a2pk: [128, BL, A] bf16 with row 0 = attn2[b], rows 1.. = 0
            a2pk = consts.tile([128, BL, A], BF16)
            nc.gpsimd.memset(a2pk[:], 0.0)
            nc.scalar.copy(a2pk[0:1, :, :], a2_flat[:])

            # ---------------- persistent accumulators ----------------
            w_all = persist.tile([128, NT, BL], F32)
            # context rows [1, 512] at partition 32*(b%4) of two bank-tiles
            ctx_ps = [
                psum_acc_pool.tile([128, C], F32, tag=f"ctx{i}", name=f"ctx_ps{i}") for i in range(2)
            ]
            z_ps = psum_acc_pool.tile([BL, 1], F32)

            # ---------------- main loop over p-tiles ----------------
            for t in range(NT):
                keysN = keysN_pool.tile([PT, BL, C], BF16)
                nc.sync.dma_start(keysN[:], keysN_d[t])
                keysT = keysT_pool.tile([128, BL, NC_CH, PT], BF16)
                nc.sync.dma_start(keysT[:], keysT_d[t])

                scores_t = small_pool.tile([128, BL], F32, tag="scores")
                mm1 = [
                    psum_mm1_pool.tile([PT, 4, A], F32, tag="mm1", name="mm1ps") for _ in range(2)
                ]
                for b in range(BL):
                    half, bi = divmod(b, 4)
                    out_ps = mm1[half][:, bi, :]
                    for mc in range(NC_CH):
                        nc.tensor.matmul(
                            out_ps,
                            keysT[:, b, mc, :],
                            waT[:, mc, :],
                            start=(bi % 2 == 0 and mc == 0),
                            stop=False,
                        )
                    # attn2 add: K=128 stationary with single 1-row, rhs row0=attn2
                    nc.tensor.matmul(
                        out_ps,
                        a2pad[:],
                        a2pk[:, b, :],
                        start=False,
                        stop=(bi % 2 == 1),
                    )
                for half in range(2):
                    # tanh over 4 batches at once
                    t_bf = tanh_pool.tile([PT, 4, A], BF16, tag="tanh")
                    nc.scalar.activation(t_bf[:], mm1[half][:], AF.Tanh)
                    for bi in range(4):
                        b = half * 4 + bi
                        jnk = junk_pool.tile([PT, A], BF16, tag="jnk")
                        nc.vector.scalar_tensor_tensor(
                            out=jnk[:],
                            in0=t_bf[:, bi, :],
                            scalar=1.0,
                            in1=va_bc[:],
                            op0=ALU.mult,
                            op1=ALU.mult,
                            accum_out=scores_t[:, b : b + 1],
                        )

                # exp(s + va_b) for all 8 b at once, keep f32 for output
                nc.scalar.activation(
                    w_all[:, t, :], scores_t[:], AF.Exp, bias=vab_bc[:]
                )
                exp_bf = small_pool.tile([128, BL], BF16, tag="expbf")
                nc.vector.tensor_copy(exp_bf[:], w_all[:, t, :])

                # context: ctx[b] += exp_b^T @ keysN[b]   (one [1,512] psum row per b)
                for b in range(BL):
                    half, bi = divmod(b, 4)
                    nc.tensor.matmul(
                        ctx_ps[half][32 * bi : 32 * bi + 1, :],
                        exp_bf[:, b : b + 1],
                        keysN[:, b, :],
                        start=(t == 0),
                        stop=(t == NT - 1),
                        tile_position=(0, 32 * bi),
                    )
                # Z[b] += sum_p exp
                nc.tensor.matmul(
                    z_ps[:],
                    exp_bf[:],
                    ones_col_bf[:],
                    start=(t == 0),
                    stop=(t == NT - 1),
                )

            # ---------------- finalize ----------------
            z_sb = setup.tile([BL, 1], F32, tag="zsb")
            nc.scalar.copy(z_sb[:], z_ps[:])
            rz = setup.tile([BL, 1], F32, tag="rz")
            nc.vector.reciprocal(rz[:], z_sb[:])

            # context rows: psum [32*bi] rows -> sbuf -> gather to [BL, C]
            ctx_sb = setup.tile([128, 2, C], F32, tag="ctxsb")
            for b in range(BL):
                half, bi = divmod(b, 4)
                nc.scalar.copy(
                    ctx_sb[32 * bi : 32 * bi + 1, half, :],
                    ctx_ps[half][32 * bi : 32 * bi + 1, :],
                )
            ctx_f = setup.tile([BL, C], F32, tag="ctxf")
            for b in range(BL):
                half, bi = divmod(b, 4)
                nc.sync.dma_start(
                    ctx_f[b : b + 1, :], ctx_sb[32 * bi : 32 * bi + 1, half, :]
                )
            nc.vector.tensor_scalar_mul(ctx_f[:], ctx_f[:], rz[:])
            nc.sync.dma_start(ctx_d[:], ctx_f[:])

            # weights: w_all[p, t, b] * rz[b] (broadcast), DMA out
            rz_row = setup.tile([1, BL], F32, tag="rzrow")
            for b in range(BL):
                nc.sync.dma_start(rz_row[0:1, b : b + 1], rz[b : b + 1, :])
            rz_bc = setup.tile([128, BL], F32, tag="rzbc")
            nc.gpsimd.partition_broadcast(rz_bc[:], rz_row[:])
            for t in range(NT):
                nc.vector.tensor_mul(w_all[:, t, :], w_all[:, t, :], rz_bc[:])
            nc.sync.dma_start(
                w_d.ap().rearrange("(t p) b -> p t b", p=PT), w_all[:]
            )

    nc.compile()
    return nc


_NC_CACHE = None


def _get_nc():
    global _NC_CACHE
    if _NC_CACHE is None:
        _NC_CACHE = build_nc()
    return _NC_CACHE


def make_in_maps(inputs):
    """Host-side prep: bf16 keys in natural and transposed tile layouts, per core."""
    keys = np.asarray(inputs["keys"])
    keys_bf = keys.astype(ml_dtypes.bfloat16)
    # keysN [NT, PT, B, C]
    keysN = keys_bf.reshape(NT, PT, B, C)
    # keysT [NT, cc, B, mc, p]: keysT[t, cc, b, mc, p] = keys[t*128+p, b, mc*128+cc]
    keysT = np.ascontiguousarray(
        keys_bf.reshape(NT, PT, B, NC_CH, 128).transpose(0, 4, 2, 3, 1)
    )
    rep = ("Wa_w", "Wa_b", "Ua_w", "Ua_b", "va_w", "va_b")
    in_maps = []
    for m in range(NCORES):
        sl = slice(m * BL, (m + 1) * BL)
        im = {
            "keysN": np.ascontiguousarray(keysN[:, :, sl, :]),
            "keysT": np.ascontiguousarray(keysT[:, :, sl, :, :]),
            "queries": np.ascontiguousarray(np.asarray(inputs["queries"])[:, sl, :]),
        }
        for k in rep:
            im[k] = np.asarray(inputs[k])
        in_maps.append(im)
    return in_maps


def kernel(**inputs):
    nc = _get_nc()
    in_maps = make_in_maps(inputs)
    res = bass_utils.run_bass_kernel_spmd(nc, in_maps, core_ids=list(range(NCORES)))
    ctx = np.zeros((1, B, C), np.float32)
    w = np.zeros((P, B, 1), np.float32)
    for m in range(NCORES):
        sl = slice(m * BL, (m + 1) * BL)
        ctx[0, sl, :] = res.results[m]["ctx_out"]
        w[:, sl, 0] = res.results[m]["w_out"]
    return ctx, w


# revision 11
# speedup vs baseline: 1.1972x; 1.1972x over previous
"""Bahdanau-style attention kernel for Trainium2, data-parallel over batch B on 8 NeuronCores.

Reference computation (per batch b):
  attn1[p,a] = sum_c keys[p,b,c] * Wa_w[a,c] + Wa_b[a]
  attn2[a]   = sum_h queries[0,b,h] * Ua_w[a,h] + Ua_b[a]
  scores[p]  = sum_a tanh(attn1[p,a] + attn2[a]) * va_w[0,a] + va_b[0]
  weights    = softmax(scores over p)
  context[c] = sum_p weights[p] * keys[p,b,c]

Scores are tiny (|s| < ~4), so softmax is computed unnormalized: exp(s) is
accumulated into Z and context_unnorm in a single pass over keys, then
normalized at the end.

The PE contracts over the partition axis, so attn1 (contraction over c) needs
keys with c on partitions while the context accumulation (contraction over p)
needs p on partitions.  Both layouts are prepared host-side in bf16 — the
device reads 2 MB per 128-row tile (same bytes as the original f32 keys read
once) and does no on-device transposes or casts.
"""

import sys

sys.path.insert(0, "/opt/trn_rl_repo")

import ml_dtypes
import numpy as np

from concourse import bacc, bass, masks, mybir, tile
from concourse import bass_utils

P, B, C, H, A = 4096, 64, 512, 512, 256
NCORES = 8
BL = B // NCORES  # 8 batches per core
PT = 128  # rows per p-tile
NT = P // PT  # 32 p-tiles
NC_CH = C // 128  # 4 contraction chunks of 128
NH_CH = H // 128
F32 = mybir.dt.float32
BF16 = mybir.dt.bfloat16
FP8 = mybir.dt.float8e4
AF = mybir.ActivationFunctionType
ALU = mybir.AluOpType


def build_nc():
    nc = bacc.Bacc("TRN2", target_bir_lowering=False, debug=False)

    # host-prepared bf16 copies of keys in both layouts
    # keysN[t, p, b, c]  = keys[t*128+p, b, c]          (p on partitions)
    # keysT[t, cc, b, mc, p] = keys[t*128+p, b, mc*128+cc]  (c-chunk on partitions)
    keysN_d = nc.dram_tensor("keysN", [NT, PT, BL, C], BF16, kind="ExternalInput")
    keysT_d = nc.dram_tensor(
        "keysT", [NT, 128, BL, NC_CH, PT], FP8, kind="ExternalInput"
    )
    q_d = nc.dram_tensor("queries", [1, BL, H], F32, kind="ExternalInput")
    waw_d = nc.dram_tensor("Wa_w", [A, C], F32, kind="ExternalInput")
    wab_d = nc.dram_tensor("Wa_b", [A], F32, kind="ExternalInput")
    uaw_d = nc.dram_tensor("Ua_w", [A, H], F32, kind="ExternalInput")
    uab_d = nc.dram_tensor("Ua_b", [A], F32, kind="ExternalInput")
    vaw_d = nc.dram_tensor("va_w", [1, A], F32, kind="ExternalInput")
    vab_d = nc.dram_tensor("va_b", [1], F32, kind="ExternalInput")
    ctx_d = nc.dram_tensor("ctx_out", [BL, C], F32, kind="ExternalOutput")
    w_d = nc.dram_tensor("w_out", [P, BL], F32, kind="ExternalOutput")

    with tile.TileContext(nc) as tc:
        with (
            tc.tile_pool(name="consts", bufs=1) as consts,
            tc.tile_pool(name="setup", bufs=2) as setup,
            tc.tile_pool(name="persist", bufs=1) as persist,
            tc.tile_pool(name="keysN", bufs=4) as keysN_pool,
            tc.tile_pool(name="keysT", bufs=4) as keysT_pool,
            tc.tile_pool(name="tanh", bufs=3) as tanh_pool,
            tc.tile_pool(name="junk", bufs=2) as junk_pool,
            tc.tile_pool(name="small", bufs=4) as small_pool,
            tc.tile_pool(name="psum_tr", bufs=1, space="PSUM") as psum_tr_pool,
            tc.tile_pool(name="psum_mm1", bufs=2, space="PSUM") as psum_mm1_pool,
            tc.tile_pool(name="psum_acc", bufs=1, space="PSUM") as psum_acc_pool,
        ):
            # ---------------- constants ----------------
            ident_bf = consts.tile([128, 128], BF16)
            masks.make_identity(nc, ident_bf[:])
            ident_fp8 = consts.tile([128, 128], FP8)
            masks.make_identity(nc, ident_fp8[:])
            ones_col_bf = consts.tile([128, 1], BF16)
            nc.gpsimd.memset(ones_col_bf[:], 1.0)
            # padded-ones stationary for the attn2 add: row 0 = 1, rows 1.. = 0
            a2pad = consts.tile([128, PT], BF16)
            nc.gpsimd.memset(a2pad[:], 0.0)
            nc.gpsimd.memset(a2pad[0:1, :], 1.0)

            # va broadcast to all partitions (for the scores reduction)
            va_f32 = setup.tile([1, A], F32)
            nc.sync.dma_start(va_f32[:], vaw_d[:])
            va_bf_row = setup.tile([1, A], BF16)
            nc.scalar.copy(va_bf_row[:], va_f32[:])
            va_bc = consts.tile([128, A], BF16)
            nc.gpsimd.partition_broadcast(va_bc[:], va_bf_row[:])

            # va_b broadcast (bias for exp)
            vab_sb = setup.tile([1, 1], F32)
            nc.sync.dma_start(vab_sb[:], vab_d[:])
            vab_bc = consts.tile([128, 1], F32)
            nc.gpsimd.partition_broadcast(vab_bc[:], vab_sb[:])

            # ---------------- transpose the small weights ----------------
            def load_transposed(w_dram, n_k, tag, dt, ident):
                w_f32 = setup.tile([128, 2, n_k * 128], F32, tag="wf32")
                nc.sync.dma_start(
                    w_f32[:], w_dram.ap().rearrange("(o p) c -> p o c", p=128)
                )
                w_lp = setup.tile([128, 2, n_k * 128], BF16, tag="wbf")
                nc.scalar.copy(w_lp[:], w_f32[:])
                wT = consts.tile([128, n_k, A], dt, tag=tag)
                for mc in range(n_k):
                    ps = psum_tr_pool.tile([128, 2, 128], BF16, tag="ktr")
                    for ac in range(2):
                        nc.tensor.transpose(
                            ps[:, ac, :],
                            w_lp[:, ac, mc * 128 : (mc + 1) * 128],
                            ident_bf[:],
                        )
                    nc.scalar.copy(wT[:, mc, :], ps[:])
                return wT

            waT = load_transposed(waw_d, NC_CH, "waT", FP8, ident_fp8[:])
            uaT = load_transposed(uaw_d, NH_CH, "uaT", BF16, ident_bf[:])

            # queries -> qT [128(hh), NH_CH, BL] bf16
            q_f32 = setup.tile([BL, H], F32)
            nc.sync.dma_start(q_f32[:], q_d.ap().rearrange("q b h -> (q b) h"))
            q_bf = setup.tile([BL, H], BF16)
            nc.scalar.copy(q_bf[:], q_f32[:])
            qT = setup.tile([128, NH_CH, BL], BF16)
            for hc in range(NH_CH):
                ps = psum_tr_pool.tile([128, BL], BF16, tag="ktr")
                nc.tensor.transpose(
                    ps[:], q_bf[:, hc * 128 : (hc + 1) * 128], ident_bf[0:BL, 0:BL]
                )
                nc.scalar.copy(qT[:, hc, :], ps[:])

            # bias_sum = Wa_b + Ua_b  [1, A]
            wab_sb = setup.tile([1, A], F32, tag="bias")
            nc.sync.dma_start(wab_sb[:], wab_d.ap())
            uab_sb = setup.tile([1, A], F32, tag="bias")
            nc.sync.dma_start(uab_sb[:], uab_d.ap())
            bias_bf = setup.tile([1, A], BF16, tag="biasbf")
            nc.vector.tensor_add(bias_bf[:], wab_sb[:], uab_sb[:])

            # attn2[b, a] = q[b] @ Ua^T + Ua_b + Wa_b   -> [BL, A] psum
            ones_row_bf = setup.tile([1, 128], BF16, tag="onesrow")
            nc.gpsimd.memset(ones_row_bf[:], 1.0)
            a2_ps = psum_mm1_pool.tile([BL, A], F32, tag="mm1")
            for hc in range(NH_CH):
                nc.tensor.matmul(
                    a2_ps[:],
                    qT[:, hc, :],
                    uaT[:, hc, :],
                    start=(hc == 0),
                    stop=False,
                )
            nc.tensor.matmul(
                a2_ps[:], ones_row_bf[0:1, 0:BL], bias_bf[:], start=False, stop=True
            )
            a2_sb = setup.tile([BL, A], F32)
            nc.scalar.copy(a2_sb[:], a2_ps[:])
            # move the 8 rows to partition 0, then bf16, rows 1..127 zero
            a2_flat = setup.tile([1, BL, A], F32)
            for b in range(BL):
                nc.sync.dma_start(a2_flat[0:1, b, :], a2_sb[b : b + 1, :])
            # a2pk: [128, BL, A] bf16 with row 0 = attn2[b], rows 1.. = 0
            a2pk = consts.tile([128, BL, A], BF16)
            nc.gpsimd.memset(a2pk[:], 0.0)
            nc.scalar.copy(a2pk[0:1, :, :], a2_flat[:])

            # ---------------- persistent accumulators ----------------
            w_all = persist.tile([128, NT, BL], F32)
            # context rows [1, 512] at partition 32*(b%4) of two bank-tiles
            ctx_ps = [
                psum_acc_pool.tile([128, C], F32, tag=f"ctx{i}", name=f"ctx_ps{i}") for i in range(2)
            ]
            z_ps = psum_acc_pool.tile([BL, 1], F32)

            # ---------------- main loop over p-tiles ----------------
            for t in range(NT):
                keysN = keysN_pool.tile([PT, BL, C], BF16)
                nc.sync.dma_start(keysN[:], keysN_d[t])
                keysT = keysT_pool.tile([128, BL, NC_CH, PT], FP8)
                nc.sync.dma_start(keysT[:], keysT_d[t])

                scores_t = small_pool.tile([128, BL], F32, tag="scores")
                mm1 = [
                    psum_mm1_pool.tile([PT, 4, A], F32, tag="mm1", name="mm1ps") for _ in range(2)
                ]
                for b in range(BL):
                    half, bi = divmod(b, 4)
                    out_ps = mm1[half][:, bi, :]
                    for mc in range(NC_CH):
                        nc.tensor.matmul(
                            out_ps,
                            keysT[:, b, mc, :],
                            waT[:, mc, :],
                            start=(bi % 2 == 0 and mc == 0),
                            stop=False,
                        )
                    # attn2 add: K=128 stationary with single 1-row, rhs row0=attn2
                    nc.tensor.matmul(
                        out_ps,
                        a2pad[:],
                        a2pk[:, b, :],
                        start=False,
                        stop=(bi % 2 == 1),
                    )
                for half in range(2):
                    # tanh over 4 batches at once
                    t_bf = tanh_pool.tile([PT, 4, A], BF16, tag="tanh")
                    nc.scalar.activation(t_bf[:], mm1[half][:], AF.Tanh)
                    for bi in range(4):
                        b = half * 4 + bi
                        jnk = junk_pool.tile([PT, A], BF16, tag="jnk")
                        nc.vector.scalar_tensor_tensor(
                            out=jnk[:],
                            in0=t_bf[:, bi, :],
                            scalar=1.0,
                            in1=va_bc[:],
                            op0=ALU.mult,
                            op1=ALU.mult,
                            accum_out=scores_t[:, b : b + 1],
                        )

                # exp(s + va_b) for all 8 b at once, keep f32 for output
                nc.scalar.activation(
                    w_all[:, t, :], scores_t[:], AF.Exp, bias=vab_bc[:]
                )
                exp_bf = small_pool.tile([128, BL], BF16, tag="expbf")
                nc.vector.tensor_copy(exp_bf[:], w_all[:, t, :])

                # context: ctx[b] += exp_b^T @ keysN[b]   (one [1,512] psum row per b)
                for b in range(BL):
                    half, bi = divmod(b, 4)
                    nc.tensor.matmul(
                        ctx_ps[half][32 * bi : 32 * bi + 1, :],
                        exp_bf[:, b : b + 1],
                        keysN[:, b, :],
                        start=(t == 0),
                        stop=(t == NT - 1),
                        tile_position=(0, 32 * bi),
                    )
                # Z[b] += sum_p exp
                nc.tensor.matmul(
                    z_ps[:],
                    exp_bf[:],
                    ones_col_bf[:],
                    start=(t == 0),
                    stop=(t == NT - 1),
                )

            # ---------------- finalize ----------------
            z_sb = setup.tile([BL, 1], F32, tag="zsb")
            nc.scalar.copy(z_sb[:], z_ps[:])
            rz = setup.tile([BL, 1], F32, tag="rz")
            nc.vector.reciprocal(rz[:], z_sb[:])

            # context rows: psum [32*bi] rows -> sbuf -> gather to [BL, C]
            ctx_sb = setup.tile([128, 2, C], F32, tag="ctxsb")
            for b in range(BL):
                half, bi = divmod(b, 4)
                nc.scalar.copy(
                    ctx_sb[32 * bi : 32 * bi + 1, half, :],
                    ctx_ps[half][32 * bi : 32 * bi + 1, :],
                )
            ctx_f = setup.tile([BL, C], F32, tag="ctxf")
            for b in range(BL):
                half, bi = divmod(b, 4)
                nc.sync.dma_start(
                    ctx_f[b : b + 1, :], ctx_sb[32 * bi : 32 * bi + 1, half, :]
                )
            nc.vector.tensor_scalar_mul(ctx_f[:], ctx_f[:], rz[:])
            nc.sync.dma_start(ctx_d[:], ctx_f[:])

            # weights: w_all[p, t, b] * rz[b] (broadcast), DMA out
            rz_row = setup.tile([1, BL], F32, tag="rzrow")
            for b in range(BL):
                nc.sync.dma_start(rz_row[0:1, b : b + 1], rz[b : b + 1, :])
            rz_bc = setup.tile([128, BL], F32, tag="rzbc")
            nc.gpsimd.partition_broadcast(rz_bc[:], rz_row[:])
            for t in range(NT):
                nc.vector.tensor_mul(w_all[:, t, :], w_all[:, t, :], rz_bc[:])
            nc.sync.dma_start(
                w_d.ap().rearrange("(t p) b -> p t b", p=PT), w_all[:]
            )

    nc.compile()
    return nc


_NC_CACHE = None


def _get_nc():
    global _NC_CACHE
    if _NC_CACHE is None:
        _NC_CACHE = build_nc()
    return _NC_CACHE


def make_in_maps(inputs):
    """Host-side prep: bf16 keys in natural and transposed tile layouts, per core."""
    keys = np.asarray(inputs["keys"])
    keys_bf = keys.astype(ml_dtypes.bfloat16)
    # keysN [NT, PT, B, C]
    keysN = keys_bf.reshape(NT, PT, B, C)
    # keysT [NT, cc, B, mc, p]: keysT[t, cc, b, mc, p] = keys[t*128+p, b, mc*128+cc]
    keysT = np.ascontiguousarray(
        keys.astype(ml_dtypes.float8_e4m3)
        .reshape(NT, PT, B, NC_CH, 128)
        .transpose(0, 4, 2, 3, 1)
    )
    rep = ("Wa_w", "Wa_b", "Ua_w", "Ua_b", "va_w", "va_b")
    in_maps = []
    for m in range(NCORES):
        sl = slice(m * BL, (m + 1) * BL)
        im = {
            "keysN": np.ascontiguousarray(keysN[:, :, sl, :]),
            "keysT": np.ascontiguousarray(keysT[:, :, sl, :, :]),
            "queries": np.ascontiguousarray(np.asarray(inputs["queries"])[:, sl, :]),
        }
        for k in rep:
            im[k] = np.asarray(inputs[k])
        in_maps.append(im)
    return in_maps


def kernel(**inputs):
    nc = _get_nc()
    in_maps = make_in_maps(inputs)
    res = bass_utils.run_bass_kernel_spmd(nc, in_maps, core_ids=list(range(NCORES)))
    ctx = np.zeros((1, B, C), np.float32)
    w = np.zeros((P, B, 1), np.float32)
    for m in range(NCORES):
        sl = slice(m * BL, (m + 1) * BL)
        ctx[0, sl, :] = res.results[m]["ctx_out"]
        w[:, sl, 0] = res.results[m]["w_out"]
    return ctx, w


# revision 12
# speedup vs baseline: 1.2642x; 1.0560x over previous
"""Bahdanau-style attention kernel for Trainium2, data-parallel over batch B on 8 NeuronCores.

Reference computation (per batch b):
  attn1[p,a] = sum_c keys[p,b,c] * Wa_w[a,c] + Wa_b[a]
  attn2[a]   = sum_h queries[0,b,h] * Ua_w[a,h] + Ua_b[a]
  scores[p]  = sum_a tanh(attn1[p,a] + attn2[a]) * va_w[0,a] + va_b[0]
  weights    = softmax(scores over p)
  context[c] = sum_p weights[p] * keys[p,b,c]

Scores are tiny (|s| < ~4), so softmax is computed unnormalized: exp(s) is
accumulated into Z and context_unnorm in a single pass over keys, then
normalized at the end.

The PE contracts over the partition axis, so attn1 (contraction over c) needs
keys with c on partitions while the context accumulation (contraction over p)
needs p on partitions.  Both layouts are prepared host-side: keysT in fp8
(feeds only the scores path, where quantization error averages out over the
C=512 contraction) and keysN in bf16 (feeds the context accumulation).  The
device reads 1.5 MB per 128-row tile and does no transposes or casts of keys.
attn1 runs as fp8 DoubleRow matmuls (2 MACs/cell/cycle).  The small weight
operands (WaT, UaT, qT) are also laid out host-side.
"""

import sys

sys.path.insert(0, "/opt/trn_rl_repo")

import ml_dtypes
import numpy as np

from concourse import bacc, bass, masks, mybir, tile
from concourse import bass_utils

P, B, C, H, A = 4096, 64, 512, 512, 256
NCORES = 8
BL = B // NCORES  # 8 batches per core
PT = 128  # rows per p-tile
NT = P // PT  # 32 p-tiles
NC_CH = C // 128  # 4 contraction chunks of 128
NH_CH = H // 128
F32 = mybir.dt.float32
BF16 = mybir.dt.bfloat16
FP8 = mybir.dt.float8e4
AF = mybir.ActivationFunctionType
ALU = mybir.AluOpType
PM = mybir.MatmulPerfMode


def build_nc():
    nc = bacc.Bacc("TRN2", target_bir_lowering=False, debug=False)

    # host-prepared key layouts
    # keysN[t, p, b, c]      = keys[t*128+p, b, c]            bf16 (p on partitions)
    # keysT[t, cc, b, mc, p] = keys[t*128+p, b, mc*128+cc]    fp8  (c-chunk on partitions)
    keysN_d = nc.dram_tensor("keysN", [NT, PT, BL, C], BF16, kind="ExternalInput")
    keysT_d = nc.dram_tensor(
        "keysT", [NT, 128, BL, NC_CH, PT], FP8, kind="ExternalInput"
    )
    # host-prepared weight layouts
    # waT[cc, mc, a] = Wa_w[a, mc*128+cc] fp8 ; uaT likewise bf16 ; qT[hh, hc, b] bf16
    waT_d = nc.dram_tensor("waT_h", [128, NC_CH, A], FP8, kind="ExternalInput")
    uaT_d = nc.dram_tensor("uaT_h", [128, NH_CH, A], BF16, kind="ExternalInput")
    qT_d = nc.dram_tensor("qT_h", [128, NH_CH, BL], BF16, kind="ExternalInput")
    wab_d = nc.dram_tensor("Wa_b", [A], F32, kind="ExternalInput")
    uab_d = nc.dram_tensor("Ua_b", [A], F32, kind="ExternalInput")
    vaw_d = nc.dram_tensor("va_w", [1, A], F32, kind="ExternalInput")
    vab_d = nc.dram_tensor("va_b", [1], F32, kind="ExternalInput")
    ctx_d = nc.dram_tensor("ctx_out", [BL, C], F32, kind="ExternalOutput")
    w_d = nc.dram_tensor("w_out", [P, BL], F32, kind="ExternalOutput")

    with tile.TileContext(nc) as tc:
        with (
            tc.tile_pool(name="consts", bufs=1) as consts,
            tc.tile_pool(name="setup", bufs=2) as setup,
            tc.tile_pool(name="persist", bufs=1) as persist,
            tc.tile_pool(name="keysN", bufs=4) as keysN_pool,
            tc.tile_pool(name="keysT", bufs=4) as keysT_pool,
            tc.tile_pool(name="tanh", bufs=3) as tanh_pool,
            tc.tile_pool(name="junk", bufs=2) as junk_pool,
            tc.tile_pool(name="small", bufs=4) as small_pool,
            tc.tile_pool(name="psum_a2", bufs=1, space="PSUM") as psum_a2_pool,
            tc.tile_pool(name="psum_mm1", bufs=2, space="PSUM") as psum_mm1_pool,
            tc.tile_pool(name="psum_acc", bufs=1, space="PSUM") as psum_acc_pool,
        ):
            # ---------------- constants / small weights ----------------
            waT = consts.tile([128, NC_CH, A], FP8)
            nc.sync.dma_start(waT[:], waT_d[:])
            uaT = consts.tile([128, NH_CH, A], BF16)
            nc.sync.dma_start(uaT[:], uaT_d[:])
            qT = consts.tile([128, NH_CH, BL], BF16)
            nc.sync.dma_start(qT[:], qT_d[:])

            ones_col_bf = consts.tile([128, 1], BF16)
            nc.gpsimd.memset(ones_col_bf[:], 1.0)
            # padded-ones stationary for the attn2 add: row 0 = 1, rows 1.. = 0
            a2pad = consts.tile([128, PT], BF16)
            nc.gpsimd.memset(a2pad[:], 0.0)
            nc.gpsimd.memset(a2pad[0:1, :], 1.0)

            # va broadcast to all partitions (for the scores reduction)
            va_f32 = setup.tile([1, A], F32)
            nc.sync.dma_start(va_f32[:], vaw_d[:])
            va_bf_row = setup.tile([1, A], BF16)
            nc.scalar.copy(va_bf_row[:], va_f32[:])
            va_bc = consts.tile([128, A], BF16)
            nc.gpsimd.partition_broadcast(va_bc[:], va_bf_row[:])

            # va_b broadcast (bias for exp)
            vab_sb = setup.tile([1, 1], F32)
            nc.sync.dma_start(vab_sb[:], vab_d[:])
            vab_bc = consts.tile([128, 1], F32)
            nc.gpsimd.partition_broadcast(vab_bc[:], vab_sb[:])

            # bias_sum = Wa_b + Ua_b  [1, A]
            wab_sb = setup.tile([1, A], F32, tag="bias")
            nc.sync.dma_start(wab_sb[:], wab_d.ap())
            uab_sb = setup.tile([1, A], F32, tag="bias")
            nc.sync.dma_start(uab_sb[:], uab_d.ap())
            bias_bf = setup.tile([1, A], BF16, tag="biasbf")
            nc.vector.tensor_add(bias_bf[:], wab_sb[:], uab_sb[:])

            # attn2[b, a] = q[b] @ Ua^T + Ua_b + Wa_b   -> [BL, A] psum
            ones_row_bf = setup.tile([1, 128], BF16, tag="onesrow")
            nc.gpsimd.memset(ones_row_bf[:], 1.0)
            a2_ps = psum_a2_pool.tile([BL, A], F32)
            for hc in range(NH_CH):
                nc.tensor.matmul(
                    a2_ps[:],
                    qT[:, hc, :],
                    uaT[:, hc, :],
                    start=(hc == 0),
                    stop=False,
                )
            nc.tensor.matmul(
                a2_ps[:], ones_row_bf[0:1, 0:BL], bias_bf[:], start=False, stop=True
            )
            a2_sb = setup.tile([BL, A], F32)
            nc.scalar.copy(a2_sb[:], a2_ps[:])
            # move the 8 rows to partition 0; a2pk row 0 = attn2[b], rows 1.. = 0
            a2_flat = setup.tile([1, BL, A], F32)
            nc.sync.dma_start(
                a2_flat[0:1, :, :], a2_sb[:, :]
            )
            a2pk = consts.tile([128, BL, A], BF16)
            nc.gpsimd.memset(a2pk[:], 0.0)
            nc.scalar.copy(a2pk[0:1, :, :], a2_flat[:])

            # ---------------- persistent accumulators ----------------
            w_all = persist.tile([128, NT, BL], F32)
            ctx_ps = [
                psum_acc_pool.tile([128, C], F32, tag=f"ctx{i}", name=f"ctx_ps{i}")
                for i in range(2)
            ]
            z_ps = psum_acc_pool.tile([BL, 1], F32)

            # ---------------- main loop over p-tiles ----------------
            for t in range(NT):
                keysN = keysN_pool.tile([PT, BL, C], BF16)
                nc.sync.dma_start(keysN[:], keysN_d[t])
                keysT = keysT_pool.tile([128, BL, NC_CH, PT], FP8)
                nc.sync.dma_start(keysT[:], keysT_d[t])

                scores_t = small_pool.tile([128, BL], F32, tag="scores")
                mm1 = [
                    psum_mm1_pool.tile([PT, 4, A], F32, tag="mm1", name="mm1ps")
                    for _ in range(2)
                ]
                for b in range(BL):
                    half, bi = divmod(b, 4)
                    out_ps = mm1[half][:, bi, :]
                    for q in range(2):
                        nc.tensor.matmul(
                            out_ps,
                            keysT[:, b, 2 * q : 2 * q + 2, :],
                            waT[:, 2 * q : 2 * q + 2, :],
                            start=(bi % 2 == 0 and q == 0),
                            stop=False,
                            perf_mode=PM.DoubleRow,
                        )
                    # attn2 add: K=128 stationary with single 1-row, rhs row0=attn2
                    nc.tensor.matmul(
                        out_ps,
                        a2pad[:],
                        a2pk[:, b, :],
                        start=False,
                        stop=(bi % 2 == 1),
                    )
                for half in range(2):
                    t_bf = tanh_pool.tile([PT, 4, A], BF16, tag="tanh")
                    nc.scalar.activation(t_bf[:], mm1[half][:], AF.Tanh)
                    for bi in range(4):
                        b = half * 4 + bi
                        jnk = junk_pool.tile([PT, A], BF16, tag="jnk")
                        nc.vector.scalar_tensor_tensor(
                            out=jnk[:],
                            in0=t_bf[:, bi, :],
                            scalar=1.0,
                            in1=va_bc[:],
                            op0=ALU.mult,
                            op1=ALU.mult,
                            accum_out=scores_t[:, b : b + 1],
                        )

                # exp(s + va_b) for all 8 b at once, keep f32 for output
                nc.scalar.activation(
                    w_all[:, t, :], scores_t[:], AF.Exp, bias=vab_bc[:]
                )
                exp_bf = small_pool.tile([128, BL], BF16, tag="expbf")
                nc.vector.tensor_copy(exp_bf[:], w_all[:, t, :])

                # context: ctx[b] += exp_b^T @ keysN[b]   (one [1,512] psum row per b)
                for b in range(BL):
                    half, bi = divmod(b, 4)
                    nc.tensor.matmul(
                        ctx_ps[half][32 * bi : 32 * bi + 1, :],
                        exp_bf[:, b : b + 1],
                        keysN[:, b, :],
                        start=(t == 0),
                        stop=(t == NT - 1),
                        tile_position=(0, 32 * bi),
                    )
                # Z[b] += sum_p exp
                nc.tensor.matmul(
                    z_ps[:],
                    exp_bf[:],
                    ones_col_bf[:],
                    start=(t == 0),
                    stop=(t == NT - 1),
                )

            # ---------------- finalize ----------------
            z_sb = setup.tile([BL, 1], F32, tag="zsb")
            nc.scalar.copy(z_sb[:], z_ps[:])
            rz = setup.tile([BL, 1], F32, tag="rz")
            nc.vector.reciprocal(rz[:], z_sb[:])

            # rz broadcast for the weights normalize
            rz_row = setup.tile([1, BL], F32, tag="rzrow")
            nc.sync.dma_start(rz_row[0:1, :], rz[:, :])
            rz_bc = setup.tile([128, BL], F32, tag="rzbc")
            nc.gpsimd.partition_broadcast(rz_bc[:], rz_row[:])

            # weights: normalize + store in 4 interleaved chunks
            w_view = w_d.ap().rearrange("(t p) b -> t p b", p=PT)
            CH = NT // 4
            for g in range(4):
                for t in range(g * CH, (g + 1) * CH):
                    nc.vector.tensor_mul(
                        w_all[:, t, :], w_all[:, t, :], rz_bc[:]
                    )
                nc.sync.dma_start(
                    w_view[g * CH : (g + 1) * CH].rearrange("t p b -> p t b"),
                    w_all[:, g * CH : (g + 1) * CH, :],
                )

            # context rows: psum [32*bi] rows -> sbuf -> gather to [BL, C]
            ctx_sb = setup.tile([128, 2, C], F32, tag="ctxsb")
            for b in range(BL):
                half, bi = divmod(b, 4)
                nc.scalar.copy(
                    ctx_sb[32 * bi : 32 * bi + 1, half, :],
                    ctx_ps[half][32 * bi : 32 * bi + 1, :],
                )
            ctx_f = setup.tile([BL, C], F32, tag="ctxf")
            for b in range(BL):
                half, bi = divmod(b, 4)
                nc.sync.dma_start(
                    ctx_f[b : b + 1, :], ctx_sb[32 * bi : 32 * bi + 1, half, :]
                )
            nc.vector.tensor_scalar_mul(ctx_f[:], ctx_f[:], rz[:])
            nc.sync.dma_start(ctx_d[:], ctx_f[:])

    nc.compile()
    return nc


_NC_CACHE = None


def _get_nc():
    global _NC_CACHE
    if _NC_CACHE is None:
        _NC_CACHE = build_nc()
    return _NC_CACHE


def make_in_maps(inputs):
    """Host-side prep: keys in natural (bf16) and transposed (fp8) tile layouts."""
    keys = np.asarray(inputs["keys"])
    keysN = keys.astype(ml_dtypes.bfloat16).reshape(NT, PT, B, C)
    keysT = np.ascontiguousarray(
        keys.astype(ml_dtypes.float8_e4m3)
        .reshape(NT, PT, B, NC_CH, 128)
        .transpose(0, 4, 2, 3, 1)
    )
    # waT[cc, mc, a] = Wa_w[a, mc*128+cc]
    waT = np.ascontiguousarray(
        np.asarray(inputs["Wa_w"])
        .astype(ml_dtypes.float8_e4m3)
        .reshape(A, NC_CH, 128)
        .transpose(2, 1, 0)
    )
    uaT = np.ascontiguousarray(
        np.asarray(inputs["Ua_w"])
        .astype(ml_dtypes.bfloat16)
        .reshape(A, NH_CH, 128)
        .transpose(2, 1, 0)
    )
    q = np.asarray(inputs["queries"])  # [1, B, H]
    qT_full = (
        q[0].astype(ml_dtypes.bfloat16).reshape(B, NH_CH, 128).transpose(2, 1, 0)
    )  # [hh, hc, b]
    rep = ("Wa_b", "Ua_b", "va_w", "va_b")
    in_maps = []
    for m in range(NCORES):
        sl = slice(m * BL, (m + 1) * BL)
        im = {
            "keysN": np.ascontiguousarray(keysN[:, :, sl, :]),
            "keysT": np.ascontiguousarray(keysT[:, :, sl, :, :]),
            "waT_h": waT,
            "uaT_h": uaT,
            "qT_h": np.ascontiguousarray(qT_full[:, :, sl]),
        }
        for k in rep:
            im[k] = np.asarray(inputs[k])
        in_maps.append(im)
    return in_maps


def kernel(**inputs):
    nc = _get_nc()
    in_maps = make_in_maps(inputs)
    res = bass_utils.run_bass_kernel_spmd(nc, in_maps, core_ids=list(range(NCORES)))
    ctx = np.zeros((1, B, C), np.float32)
    w = np.zeros((P, B, 1), np.float32)
    for m in range(NCORES):
        sl = slice(m * BL, (m + 1) * BL)
        ctx[0, sl, :] = res.results[m]["ctx_out"]
        w[:, sl, 0] = res.results[m]["w_out"]
    return ctx, w


# revision 13
# speedup vs baseline: 1.3046x; 1.0319x over previous
"""Bahdanau-style attention kernel for Trainium2, data-parallel over batch B on 8 NeuronCores.

Reference computation (per batch b):
  attn1[p,a] = sum_c keys[p,b,c] * Wa_w[a,c] + Wa_b[a]
  attn2[a]   = sum_h queries[0,b,h] * Ua_w[a,h] + Ua_b[a]
  scores[p]  = sum_a tanh(attn1[p,a] + attn2[a]) * va_w[0,a] + va_b[0]
  weights    = softmax(scores over p)
  context[c] = sum_p weights[p] * keys[p,b,c]

Scores are tiny (|s| < ~4), so softmax is computed unnormalized: exp(s) is
accumulated into Z and context_unnorm in a single pass over keys, then
normalized at the end.

The PE contracts over the partition axis, so attn1 (contraction over c) needs
keys with c on partitions while the context accumulation (contraction over p)
needs p on partitions.  Both layouts are prepared host-side: keysT in fp8
(feeds only the scores path, where quantization error averages out over the
C=512 contraction) and keysN in bf16 (feeds the context accumulation).  The
device reads 1.5 MB per 128-row tile and does no transposes or casts of keys.
attn1 runs as fp8 DoubleRow matmuls (2 MACs/cell/cycle).  The small weight
operands (WaT, UaT, qT) are also laid out host-side.
"""

import sys

sys.path.insert(0, "/opt/trn_rl_repo")

import ml_dtypes
import numpy as np

from concourse import bacc, bass, masks, mybir, tile
from concourse import bass_utils

P, B, C, H, A = 4096, 64, 512, 512, 256
NCORES = 8
BL = B // NCORES  # 8 batches per core
PT = 128  # rows per p-tile
NT = P // PT  # 32 p-tiles
NC_CH = C // 128  # 4 contraction chunks of 128
NH_CH = H // 128
F32 = mybir.dt.float32
BF16 = mybir.dt.bfloat16
FP8 = mybir.dt.float8e4
AF = mybir.ActivationFunctionType
ALU = mybir.AluOpType
PM = mybir.MatmulPerfMode


def build_nc():
    nc = bacc.Bacc("TRN2", target_bir_lowering=False, debug=False)

    # host-prepared key layouts
    # keysN[t, p, b, c]      = keys[t*128+p, b, c]            bf16 (p on partitions)
    # keysT[t, cc, b, mc, p] = keys[t*128+p, b, mc*128+cc]    fp8  (c-chunk on partitions)
    keysN_d = nc.dram_tensor("keysN", [NT, PT, BL, C], BF16, kind="ExternalInput")
    keysT_d = nc.dram_tensor(
        "keysT", [NT, 128, BL, NC_CH, PT], FP8, kind="ExternalInput"
    )
    # host-prepared weight layouts
    # waT[cc, mc, a] = Wa_w[a, mc*128+cc] fp8 ; uaT likewise bf16 ; qT[hh, hc, b] bf16
    waT_d = nc.dram_tensor("waT_h", [128, NC_CH, A], FP8, kind="ExternalInput")
    uaT_d = nc.dram_tensor("uaT_h", [128, NH_CH, A], BF16, kind="ExternalInput")
    qT_d = nc.dram_tensor("qT_h", [128, NH_CH, BL], BF16, kind="ExternalInput")
    wab_d = nc.dram_tensor("Wa_b", [A], F32, kind="ExternalInput")
    uab_d = nc.dram_tensor("Ua_b", [A], F32, kind="ExternalInput")
    vaw_d = nc.dram_tensor("va_w", [1, A], F32, kind="ExternalInput")
    vab_d = nc.dram_tensor("va_b", [1], F32, kind="ExternalInput")
    ctx_d = nc.dram_tensor("ctx_out", [BL, C], F32, kind="ExternalOutput")
    w_d = nc.dram_tensor("w_out", [P, BL], F32, kind="ExternalOutput")

    with tile.TileContext(nc) as tc:
        with (
            tc.tile_pool(name="consts", bufs=1) as consts,
            tc.tile_pool(name="setup", bufs=2) as setup,
            tc.tile_pool(name="persist", bufs=1) as persist,
            tc.tile_pool(name="keysN", bufs=4) as keysN_pool,
            tc.tile_pool(name="keysT", bufs=4) as keysT_pool,
            tc.tile_pool(name="tanh", bufs=3) as tanh_pool,
            tc.tile_pool(name="junk", bufs=2) as junk_pool,
            tc.tile_pool(name="small", bufs=4) as small_pool,
            tc.tile_pool(name="psum_a2", bufs=1, space="PSUM") as psum_a2_pool,
            tc.tile_pool(name="psum_mm1", bufs=2, space="PSUM") as psum_mm1_pool,
            tc.tile_pool(name="psum_acc", bufs=1, space="PSUM") as psum_acc_pool,
        ):
            # ---------------- constants / small weights ----------------
            waT = consts.tile([128, NC_CH, A], FP8)
            nc.sync.dma_start(waT[:], waT_d[:])
            uaT = consts.tile([128, NH_CH, A], BF16)
            nc.sync.dma_start(uaT[:], uaT_d[:])
            qT = consts.tile([128, NH_CH, BL], BF16)
            nc.sync.dma_start(qT[:], qT_d[:])

            ones_col_f32 = consts.tile([128, 1], F32)
            nc.vector.memset(ones_col_f32[:], 1.0)
            # padded-ones stationary for the attn2 add: row 0 = 1, rows 1.. = 0
            a2pad = consts.tile([128, PT], BF16)
            nc.vector.memset(a2pad[:], 0.0)
            nc.vector.memset(a2pad[0:1, :], 1.0)

            # va broadcast to all partitions (for the scores reduction)
            va_f32 = setup.tile([1, A], F32)
            nc.sync.dma_start(va_f32[:], vaw_d[:])
            va_bf_row = setup.tile([1, A], BF16)
            nc.scalar.copy(va_bf_row[:], va_f32[:])
            va_bc = consts.tile([128, A], BF16)
            nc.gpsimd.partition_broadcast(va_bc[:], va_bf_row[:])

            # va_b broadcast (bias for exp)
            vab_sb = setup.tile([1, 1], F32)
            nc.sync.dma_start(vab_sb[:], vab_d[:])
            vab_bc = consts.tile([128, 1], F32)
            nc.gpsimd.partition_broadcast(vab_bc[:], vab_sb[:])

            # bias_sum = Wa_b + Ua_b  [1, A]
            wab_sb = setup.tile([1, A], F32, tag="bias")
            nc.sync.dma_start(wab_sb[:], wab_d.ap())
            uab_sb = setup.tile([1, A], F32, tag="bias")
            nc.sync.dma_start(uab_sb[:], uab_d.ap())
            bias_bf = setup.tile([1, A], BF16, tag="biasbf")
            nc.vector.tensor_add(bias_bf[:], wab_sb[:], uab_sb[:])

            # attn2[b, a] = q[b] @ Ua^T + Ua_b + Wa_b   -> [BL, A] psum
            ones_row_bf = setup.tile([1, 128], BF16, tag="onesrow")
            nc.gpsimd.memset(ones_row_bf[:], 1.0)
            a2_ps = psum_a2_pool.tile([BL, A], F32)
            for hc in range(NH_CH):
                nc.tensor.matmul(
                    a2_ps[:],
                    qT[:, hc, :],
                    uaT[:, hc, :],
                    start=(hc == 0),
                    stop=False,
                )
            nc.tensor.matmul(
                a2_ps[:], ones_row_bf[0:1, 0:BL], bias_bf[:], start=False, stop=True
            )
            a2_sb = setup.tile([BL, A], F32)
            nc.scalar.copy(a2_sb[:], a2_ps[:])
            # move the 8 rows to partition 0; a2pk row 0 = attn2[b], rows 1.. = 0
            a2_flat = setup.tile([1, BL, A], F32)
            nc.sync.dma_start(
                a2_flat[0:1, :, :], a2_sb[:, :]
            )
            a2pk = consts.tile([128, BL, A], BF16)
            nc.vector.memset(a2pk[:], 0.0)
            nc.scalar.copy(a2pk[0:1, :, :], a2_flat[:])

            # ---------------- persistent accumulators ----------------
            w_all = persist.tile([128, NT, BL], F32)
            ctx_ps = [
                psum_acc_pool.tile([128, C], F32, tag=f"ctx{i}", name=f"ctx_ps{i}")
                for i in range(2)
            ]
            z_ps = psum_acc_pool.tile([BL, 1], F32)
            for i in range(2):
                nc.vector.memset(ctx_ps[i][:], 0.0)

            # ---------------- main loop over p-tiles ----------------
            for t in range(NT):
                keysN = keysN_pool.tile([PT, BL, C], BF16)
                nc.sync.dma_start(keysN[:], keysN_d[t])
                keysT = keysT_pool.tile([128, BL, NC_CH, PT], FP8)
                nc.sync.dma_start(keysT[:], keysT_d[t])

                mm1 = [
                    psum_mm1_pool.tile([PT, 4, A], F32, tag="mm1", name="mm1ps")
                    for _ in range(2)
                ]
                for b in range(BL):
                    half, bi = divmod(b, 4)
                    out_ps = mm1[half][:, bi, :]
                    for q in range(2):
                        nc.tensor.matmul(
                            out_ps,
                            keysT[:, b, 2 * q : 2 * q + 2, :],
                            waT[:, 2 * q : 2 * q + 2, :],
                            start=(bi % 2 == 0 and q == 0),
                            stop=False,
                            perf_mode=PM.DoubleRow,
                        )
                    # attn2 add: K=128 stationary with single 1-row, rhs row0=attn2
                    nc.tensor.matmul(
                        out_ps,
                        a2pad[:],
                        a2pk[:, b, :],
                        start=False,
                        stop=(bi % 2 == 1),
                    )
                for half in range(2):
                    t_bf = tanh_pool.tile([PT, 4, A], BF16, tag="tanh")
                    nc.scalar.activation(t_bf[:], mm1[half][:], AF.Tanh)
                    scores_h = small_pool.tile([128, 4], F32, tag="scores")
                    for bi in range(4):
                        jnk = junk_pool.tile([PT, A], BF16, tag="jnk")
                        nc.vector.scalar_tensor_tensor(
                            out=jnk[:],
                            in0=t_bf[:, bi, :],
                            scalar=1.0,
                            in1=va_bc[:],
                            op0=ALU.mult,
                            op1=ALU.mult,
                            accum_out=scores_h[:, bi : bi + 1],
                        )
                    # exp(s + va_b) for this half, keep f32 for output
                    nc.scalar.activation(
                        w_all[:, t, 4 * half : 4 * half + 4],
                        scores_h[:],
                        AF.Exp,
                        bias=vab_bc[:],
                    )
                    exp_bf = small_pool.tile([128, 4], BF16, tag="expbf")
                    nc.vector.tensor_copy(
                        exp_bf[:], w_all[:, t, 4 * half : 4 * half + 4]
                    )
                    # context: ctx[b] += exp_b^T @ keysN[b]  (one [1,512] psum row per b)
                    for bi in range(4):
                        b = half * 4 + bi
                        nc.tensor.matmul(
                            ctx_ps[half][32 * bi : 32 * bi + 1, :],
                            exp_bf[:, bi : bi + 1],
                            keysN[:, b, :],
                            start=(t == 0),
                            stop=(t == NT - 1),
                            tile_position=(0, 32 * bi),
                        )
                # Z[b] += sum_p exp   (f32, reads w_all directly)
                nc.tensor.matmul(
                    z_ps[:],
                    w_all[:, t, :],
                    ones_col_f32[:],
                    start=(t == 0),
                    stop=(t == NT - 1),
                )

            # ---------------- finalize ----------------
            z_sb = setup.tile([BL, 1], F32, tag="zsb")
            nc.scalar.copy(z_sb[:], z_ps[:])
            rz = setup.tile([BL, 1], F32, tag="rz")
            nc.vector.reciprocal(rz[:], z_sb[:])

            # rz broadcast for the weights normalize
            rz_row = setup.tile([1, BL], F32, tag="rzrow")
            nc.sync.dma_start(rz_row[0:1, :], rz[:, :])
            rz_bc = setup.tile([128, BL], F32, tag="rzbc")
            nc.gpsimd.partition_broadcast(rz_bc[:], rz_row[:])

            # weights: normalize + store in 4 interleaved chunks
            w_view = w_d.ap().rearrange("(t p) b -> t p b", p=PT)
            CH = NT // 4
            for g in range(4):
                for t in range(g * CH, (g + 1) * CH):
                    nc.vector.tensor_mul(
                        w_all[:, t, :], w_all[:, t, :], rz_bc[:]
                    )
                nc.sync.dma_start(
                    w_view[g * CH : (g + 1) * CH].rearrange("t p b -> p t b"),
                    w_all[:, g * CH : (g + 1) * CH, :],
                )

            # context rows: psum [32*bi] rows -> sbuf -> gather to [BL, C]
            ctx_sb = setup.tile([128, 2, C], F32, tag="ctxsb")
            for half in range(2):
                nc.scalar.copy(ctx_sb[:, half, :], ctx_ps[half][:])
            ctx_f = setup.tile([BL, C], F32, tag="ctxf")
            for half in range(2):
                nc.sync.dma_start(
                    ctx_f[4 * half : 4 * half + 4, :],
                    ctx_sb[0:128:32, half, :],
                )
            nc.vector.tensor_scalar_mul(ctx_f[:], ctx_f[:], rz[:])
            nc.sync.dma_start(ctx_d[:], ctx_f[:])

    nc.compile()
    return nc


_NC_CACHE = None


def _get_nc():
    global _NC_CACHE
    if _NC_CACHE is None:
        _NC_CACHE = build_nc()
    return _NC_CACHE


def make_in_maps(inputs):
    """Host-side prep: keys in natural (bf16) and transposed (fp8) tile layouts."""
    keys = np.asarray(inputs["keys"])
    keysN = keys.astype(ml_dtypes.bfloat16).reshape(NT, PT, B, C)
    keysT = np.ascontiguousarray(
        keys.astype(ml_dtypes.float8_e4m3)
        .reshape(NT, PT, B, NC_CH, 128)
        .transpose(0, 4, 2, 3, 1)
    )
    # waT[cc, mc, a] = Wa_w[a, mc*128+cc]
    waT = np.ascontiguousarray(
        np.asarray(inputs["Wa_w"])
        .astype(ml_dtypes.float8_e4m3)
        .reshape(A, NC_CH, 128)
        .transpose(2, 1, 0)
    )
    uaT = np.ascontiguousarray(
        np.asarray(inputs["Ua_w"])
        .astype(ml_dtypes.bfloat16)
        .reshape(A, NH_CH, 128)
        .transpose(2, 1, 0)
    )
    q = np.asarray(inputs["queries"])  # [1, B, H]
    qT_full = (
        q[0].astype(ml_dtypes.bfloat16).reshape(B, NH_CH, 128).transpose(2, 1, 0)
    )  # [hh, hc, b]
    rep = ("Wa_b", "Ua_b", "va_w", "va_b")
    in_maps = []
    for m in range(NCORES):
        sl = slice(m * BL, (m + 1) * BL)
        im = {
            "keysN": np.ascontiguousarray(keysN[:, :, sl, :]),
            "keysT": np.ascontiguousarray(keysT[:, :, sl, :, :]),
            "waT_h": waT,
            "uaT_h": uaT,
            "qT_h": np.ascontiguousarray(qT_full[:, :, sl]),
        }
        for k in rep:
            im[k] = np.asarray(inputs[k])
        in_maps.append(im)
    return in_maps


def kernel(**inputs):
    nc = _get_nc()
    in_maps = make_in_maps(inputs)
    res = bass_utils.run_bass_kernel_spmd(nc, in_maps, core_ids=list(range(NCORES)))
    ctx = np.zeros((1, B, C), np.float32)
    w = np.zeros((P, B, 1), np.float32)
    for m in range(NCORES):
        sl = slice(m * BL, (m + 1) * BL)
        ctx[0, sl, :] = res.results[m]["ctx_out"]
        w[:, sl, 0] = res.results[m]["w_out"]
    return ctx, w


# revision 14
# speedup vs baseline: 1.3288x; 1.0186x over previous
"""Bahdanau-style attention kernel for Trainium2, data-parallel over batch B on 8 NeuronCores.

Reference computation (per batch b):
  attn1[p,a] = sum_c keys[p,b,c] * Wa_w[a,c] + Wa_b[a]
  attn2[a]   = sum_h queries[0,b,h] * Ua_w[a,h] + Ua_b[a]
  scores[p]  = sum_a tanh(attn1[p,a] + attn2[a]) * va_w[0,a] + va_b[0]
  weights    = softmax(scores over p)
  context[c] = sum_p weights[p] * keys[p,b,c]

Scores are tiny (|s| < ~4), so softmax is computed unnormalized: exp(s) is
accumulated into Z and context_unnorm in a single pass over keys, then
normalized at the end.

The PE contracts over the partition axis, so attn1 (contraction over c) needs
keys with c on partitions while the context accumulation (contraction over p)
needs p on partitions.  Both layouts are prepared host-side: keysT in fp8
(feeds only the scores path, where quantization error averages out over the
C=512 contraction) and keysN in bf16 (feeds the context accumulation).  The
device reads 1.5 MB per 128-row tile and does no transposes or casts of keys.
attn1 runs as fp8 DoubleRow matmuls (2 MACs/cell/cycle).  The small weight
operands (WaT, UaT, qT) are also laid out host-side.
"""

import sys

sys.path.insert(0, "/opt/trn_rl_repo")

import ml_dtypes
import numpy as np

from concourse import bacc, bass, masks, mybir, tile
from concourse import bass_utils

P, B, C, H, A = 4096, 64, 512, 512, 256
NCORES = 8
BL = B // NCORES  # 8 batches per core
PT = 128  # rows per p-tile
NT = P // PT  # 32 p-tiles
NC_CH = C // 128  # 4 contraction chunks of 128
NH_CH = H // 128
F32 = mybir.dt.float32
BF16 = mybir.dt.bfloat16
FP8 = mybir.dt.float8e4
AF = mybir.ActivationFunctionType
ALU = mybir.AluOpType
PM = mybir.MatmulPerfMode


def build_nc():
    nc = bacc.Bacc("TRN2", target_bir_lowering=False, debug=False)

    # host-prepared key layouts
    # keysN[t, p, b, c]      = keys[t*128+p, b, c]            bf16 (p on partitions)
    # keysT[t, cc, b, mc, p] = keys[t*128+p, b, mc*128+cc]    fp8  (c-chunk on partitions)
    keysN_d = nc.dram_tensor("keysN", [NT, PT, BL, C], BF16, kind="ExternalInput")
    keysT_d = nc.dram_tensor(
        "keysT", [NT, 128, BL, NC_CH, PT], FP8, kind="ExternalInput"
    )
    # host-prepared weight layouts
    # waT[cc, mc, a] = Wa_w[a, mc*128+cc] fp8 ; uaT likewise bf16 ; qT[hh, hc, b] bf16
    waT_d = nc.dram_tensor("waT_h", [128, NC_CH, A], FP8, kind="ExternalInput")
    uaT_d = nc.dram_tensor("uaT_h", [128, NH_CH, A], BF16, kind="ExternalInput")
    qT_d = nc.dram_tensor("qT_h", [128, NH_CH, BL], BF16, kind="ExternalInput")
    wab_d = nc.dram_tensor("Wa_b", [A], F32, kind="ExternalInput")
    uab_d = nc.dram_tensor("Ua_b", [A], F32, kind="ExternalInput")
    vaw_d = nc.dram_tensor("va_w", [1, A], F32, kind="ExternalInput")
    vab_d = nc.dram_tensor("va_b", [1], F32, kind="ExternalInput")
    ctx_d = nc.dram_tensor("ctx_out", [BL, C], F32, kind="ExternalOutput")
    w_d = nc.dram_tensor("w_out", [P, BL], F32, kind="ExternalOutput")

    with tile.TileContext(nc) as tc:
        with (
            tc.tile_pool(name="consts", bufs=1) as consts,
            tc.tile_pool(name="setup", bufs=2) as setup,
            tc.tile_pool(name="persist", bufs=1) as persist,
            tc.tile_pool(name="keysN", bufs=4) as keysN_pool,
            tc.tile_pool(name="keysT", bufs=4) as keysT_pool,
            tc.tile_pool(name="tanh", bufs=3) as tanh_pool,
            tc.tile_pool(name="junk", bufs=2) as junk_pool,
            tc.tile_pool(name="small", bufs=4) as small_pool,
            tc.tile_pool(name="psum_a2", bufs=1, space="PSUM") as psum_a2_pool,
            tc.tile_pool(name="psum_mm1", bufs=2, space="PSUM") as psum_mm1_pool,
            tc.tile_pool(name="psum_acc", bufs=1, space="PSUM") as psum_acc_pool,
        ):
            # ---------------- constants / small weights ----------------
            waT = consts.tile([128, NC_CH, A], FP8)
            nc.sync.dma_start(waT[:], waT_d[:])
            uaT = consts.tile([128, NH_CH, A], BF16)
            nc.sync.dma_start(uaT[:], uaT_d[:])
            qT = consts.tile([128, NH_CH, BL], BF16)
            nc.sync.dma_start(qT[:], qT_d[:])

            ones_col_f32 = consts.tile([128, 1], F32)
            nc.vector.memset(ones_col_f32[:], 1.0)
            # padded-ones stationary for the attn2 add: row 0 = 1, rows 1.. = 0
            a2pad = consts.tile([128, PT], BF16)
            nc.vector.memset(a2pad[:], 0.0)
            nc.vector.memset(a2pad[0:1, :], 1.0)

            # va broadcast to all partitions, repeated 4x (for the scores reduction)
            va_f32 = setup.tile([1, A], F32)
            nc.sync.dma_start(va_f32[:], vaw_d[:])
            va_bf_row = setup.tile([1, 4, A], BF16)
            for r in range(4):
                nc.scalar.copy(va_bf_row[0:1, r, :], va_f32[:])
            va_bc4 = consts.tile([128, 4, A], BF16)
            nc.gpsimd.partition_broadcast(
                va_bc4[:].rearrange("p r a -> p (r a)"),
                va_bf_row[:].rearrange("p r a -> p (r a)"),
            )

            # va_b broadcast (bias for exp)
            vab_sb = setup.tile([1, 1], F32)
            nc.sync.dma_start(vab_sb[:], vab_d[:])
            vab_bc = consts.tile([128, 1], F32)
            nc.gpsimd.partition_broadcast(vab_bc[:], vab_sb[:])

            # bias_sum = Wa_b + Ua_b  [1, A]
            wab_sb = setup.tile([1, A], F32, tag="bias")
            nc.sync.dma_start(wab_sb[:], wab_d.ap())
            uab_sb = setup.tile([1, A], F32, tag="bias")
            nc.sync.dma_start(uab_sb[:], uab_d.ap())
            bias_bf = setup.tile([1, A], BF16, tag="biasbf")
            nc.vector.tensor_add(bias_bf[:], wab_sb[:], uab_sb[:])

            # attn2[b, a] = q[b] @ Ua^T + Ua_b + Wa_b   -> [BL, A] psum
            ones_row_bf = setup.tile([1, 128], BF16, tag="onesrow")
            nc.gpsimd.memset(ones_row_bf[:], 1.0)
            a2_ps = psum_a2_pool.tile([BL, A], F32)
            for hc in range(NH_CH):
                nc.tensor.matmul(
                    a2_ps[:],
                    qT[:, hc, :],
                    uaT[:, hc, :],
                    start=(hc == 0),
                    stop=False,
                )
            nc.tensor.matmul(
                a2_ps[:], ones_row_bf[0:1, 0:BL], bias_bf[:], start=False, stop=True
            )
            a2_sb = setup.tile([BL, A], F32)
            nc.scalar.copy(a2_sb[:], a2_ps[:])
            # move the 8 rows to partition 0; a2pk row 0 = attn2[b], rows 1.. = 0
            a2_flat = setup.tile([1, BL, A], F32)
            nc.sync.dma_start(
                a2_flat[0:1, :, :], a2_sb[:, :]
            )
            a2pk = consts.tile([128, BL, A], BF16)
            nc.vector.memset(a2pk[:], 0.0)
            nc.scalar.copy(a2pk[0:1, :, :], a2_flat[:])

            # ---------------- persistent accumulators ----------------
            w_all = persist.tile([128, NT, BL], F32)
            ctx_ps = [
                psum_acc_pool.tile([128, C], F32, tag=f"ctx{i}", name=f"ctx_ps{i}")
                for i in range(2)
            ]
            z_ps = psum_acc_pool.tile([BL, 1], F32)
            for i in range(2):
                nc.vector.memset(ctx_ps[i][:], 0.0)

            # ---------------- main loop over p-tiles ----------------
            pre = {}
            for t in range(2):
                kN = keysN_pool.tile([PT, BL, C], BF16, name=f"keysN_pre{t}")
                nc.sync.dma_start(kN[:], keysN_d[t])
                kT = keysT_pool.tile([128, BL, NC_CH, PT], FP8, name=f"keysT_pre{t}")
                nc.sync.dma_start(kT[:], keysT_d[t])
                pre[t] = (kN, kT)
            for t in range(NT):
                if t in pre:
                    keysN, keysT = pre.pop(t)
                else:
                    keysN = keysN_pool.tile([PT, BL, C], BF16, name="keysN", tag="keysN_pre0")
                    nc.sync.dma_start(keysN[:], keysN_d[t])
                    keysT = keysT_pool.tile([128, BL, NC_CH, PT], FP8, name="keysT", tag="keysT_pre0")
                    nc.sync.dma_start(keysT[:], keysT_d[t])

                mm1 = [
                    psum_mm1_pool.tile([PT, 4, A], F32, tag="mm1", name="mm1ps")
                    for _ in range(2)
                ]
                for b in range(BL):
                    half, bi = divmod(b, 4)
                    out_ps = mm1[half][:, bi, :]
                    for q in range(2):
                        nc.tensor.matmul(
                            out_ps,
                            keysT[:, b, 2 * q : 2 * q + 2, :],
                            waT[:, 2 * q : 2 * q + 2, :],
                            start=(bi % 2 == 0 and q == 0),
                            stop=False,
                            perf_mode=PM.DoubleRow,
                        )
                    # attn2 add: K=128 stationary with single 1-row, rhs row0=attn2
                    nc.tensor.matmul(
                        out_ps,
                        a2pad[:],
                        a2pk[:, b, :],
                        start=False,
                        stop=(bi % 2 == 1),
                    )
                for half in range(2):
                    t_bf = tanh_pool.tile([PT, 4, A], BF16, tag="tanh")
                    nc.scalar.activation(t_bf[:], mm1[half][:], AF.Tanh)
                    prod = junk_pool.tile([PT, 4, A], BF16, tag="jnk")
                    nc.vector.tensor_mul(prod[:], t_bf[:], va_bc4[:])
                    scores_h = small_pool.tile([128, 4], F32, tag="scores")
                    nc.vector.tensor_reduce(
                        scores_h[:], prod[:], axis=mybir.AxisListType.X, op=ALU.add
                    )
                    # exp(s + va_b) for this half, keep f32 for output
                    nc.scalar.activation(
                        w_all[:, t, 4 * half : 4 * half + 4],
                        scores_h[:],
                        AF.Exp,
                        bias=vab_bc[:],
                    )
                    exp_bf = small_pool.tile([128, 4], BF16, tag="expbf")
                    nc.vector.tensor_copy(
                        exp_bf[:], w_all[:, t, 4 * half : 4 * half + 4]
                    )
                    # context: ctx[b] += exp_b^T @ keysN[b]  (one [1,512] psum row per b)
                    for bi in range(4):
                        b = half * 4 + bi
                        nc.tensor.matmul(
                            ctx_ps[half][32 * bi : 32 * bi + 1, :],
                            exp_bf[:, bi : bi + 1],
                            keysN[:, b, :],
                            start=(t == 0),
                            stop=(t == NT - 1),
                            tile_position=(0, 32 * bi),
                        )
                # Z[b] += sum_p exp   (f32, reads w_all directly)
                nc.tensor.matmul(
                    z_ps[:],
                    w_all[:, t, :],
                    ones_col_f32[:],
                    start=(t == 0),
                    stop=(t == NT - 1),
                )

            # ---------------- finalize ----------------
            z_sb = setup.tile([BL, 1], F32, tag="zsb")
            nc.scalar.copy(z_sb[:], z_ps[:])
            rz = setup.tile([BL, 1], F32, tag="rz")
            nc.vector.reciprocal(rz[:], z_sb[:])

            # rz broadcast for the weights normalize
            rz_row = setup.tile([1, BL], F32, tag="rzrow")
            nc.sync.dma_start(rz_row[0:1, :], rz[:, :])
            rz_bc = setup.tile([128, BL], F32, tag="rzbc")
            nc.gpsimd.partition_broadcast(rz_bc[:], rz_row[:])

            # weights: normalize + store in 4 interleaved chunks
            w_view = w_d.ap().rearrange("(t p) b -> t p b", p=PT)
            CH = NT // 4
            for g in range(4):
                for t in range(g * CH, (g + 1) * CH):
                    nc.vector.tensor_mul(
                        w_all[:, t, :], w_all[:, t, :], rz_bc[:]
                    )
                nc.sync.dma_start(
                    w_view[g * CH : (g + 1) * CH].rearrange("t p b -> p t b"),
                    w_all[:, g * CH : (g + 1) * CH, :],
                )

            # context rows: psum [32*bi] rows -> sbuf -> gather to [BL, C]
            ctx_sb = setup.tile([128, 2, C], F32, tag="ctxsb")
            for half in range(2):
                nc.scalar.copy(ctx_sb[:, half, :], ctx_ps[half][:])
            ctx_f = setup.tile([BL, C], F32, tag="ctxf")
            for half in range(2):
                nc.sync.dma_start(
                    ctx_f[4 * half : 4 * half + 4, :],
                    ctx_sb[0:128:32, half, :],
                )
            nc.vector.tensor_scalar_mul(ctx_f[:], ctx_f[:], rz[:])
            nc.sync.dma_start(ctx_d[:], ctx_f[:])

    nc.compile()
    return nc


_NC_CACHE = None


def _get_nc():
    global _NC_CACHE
    if _NC_CACHE is None:
        _NC_CACHE = build_nc()
    return _NC_CACHE


def make_in_maps(inputs):
    """Host-side prep: keys in natural (bf16) and transposed (fp8) tile layouts."""
    keys = np.asarray(inputs["keys"])
    keysN = keys.astype(ml_dtypes.bfloat16).reshape(NT, PT, B, C)
    keysT = np.ascontiguousarray(
        keys.astype(ml_dtypes.float8_e4m3)
        .reshape(NT, PT, B, NC_CH, 128)
        .transpose(0, 4, 2, 3, 1)
    )
    # waT[cc, mc, a] = Wa_w[a, mc*128+cc]
    waT = np.ascontiguousarray(
        np.asarray(inputs["Wa_w"])
        .astype(ml_dtypes.float8_e4m3)
        .reshape(A, NC_CH, 128)
        .transpose(2, 1, 0)
    )
    uaT = np.ascontiguousarray(
        np.asarray(inputs["Ua_w"])
        .astype(ml_dtypes.bfloat16)
        .reshape(A, NH_CH, 128)
        .transpose(2, 1, 0)
    )
    q = np.asarray(inputs["queries"])  # [1, B, H]
    qT_full = (
        q[0].astype(ml_dtypes.bfloat16).reshape(B, NH_CH, 128).transpose(2, 1, 0)
    )  # [hh, hc, b]
    rep = ("Wa_b", "Ua_b", "va_w", "va_b")
    in_maps = []
    for m in range(NCORES):
        sl = slice(m * BL, (m + 1) * BL)
        im = {
            "keysN": np.ascontiguousarray(keysN[:, :, sl, :]),
            "keysT": np.ascontiguousarray(keysT[:, :, sl, :, :]),
            "waT_h": waT,
            "uaT_h": uaT,
            "qT_h": np.ascontiguousarray(qT_full[:, :, sl]),
        }
        for k in rep:
            im[k] = np.asarray(inputs[k])
        in_maps.append(im)
    return in_maps


def kernel(**inputs):
    nc = _get_nc()
    in_maps = make_in_maps(inputs)
    res = bass_utils.run_bass_kernel_spmd(nc, in_maps, core_ids=list(range(NCORES)))
    ctx = np.zeros((1, B, C), np.float32)
    w = np.zeros((P, B, 1), np.float32)
    for m in range(NCORES):
        sl = slice(m * BL, (m + 1) * BL)
        ctx[0, sl, :] = res.results[m]["ctx_out"]
        w[:, sl, 0] = res.results[m]["w_out"]
    return ctx, w


# revision 16
# speedup vs baseline: 1.3576x; 1.0217x over previous
"""Bahdanau-style attention kernel for Trainium2, data-parallel over batch B on 8 NeuronCores.

Reference computation (per batch b):
  attn1[p,a] = sum_c keys[p,b,c] * Wa_w[a,c] + Wa_b[a]
  attn2[a]   = sum_h queries[0,b,h] * Ua_w[a,h] + Ua_b[a]
  scores[p]  = sum_a tanh(attn1[p,a] + attn2[a]) * va_w[0,a] + va_b[0]
  weights    = softmax(scores over p)
  context[c] = sum_p weights[p] * keys[p,b,c]

Scores are tiny (|s| < ~4), so softmax is computed unnormalized: exp(s) is
accumulated into Z and context_unnorm in a single pass over keys, then
normalized at the end.

The PE contracts over the partition axis, so attn1 (contraction over c) needs
keys with c on partitions while the context accumulation (contraction over p)
needs p on partitions.  Both layouts are prepared host-side: keysT in fp8
(feeds only the scores path, where quantization error averages out over the
C=512 contraction) and keysN in bf16 (feeds the context accumulation).  The
device reads 1.5 MB per 128-row tile and does no transposes or casts of keys.
attn1 runs as fp8 DoubleRow matmuls (2 MACs/cell/cycle).  The small weight
operands (WaT, UaT, qT) are also laid out host-side.
"""

import sys

sys.path.insert(0, "/opt/trn_rl_repo")

import ml_dtypes
import numpy as np

from concourse import bacc, bass, masks, mybir, tile
from concourse import bass_utils

P, B, C, H, A = 4096, 64, 512, 512, 256
NCORES = 8
BL = B // NCORES  # 8 batches per core
PT = 128  # rows per p-tile
NT = P // PT  # 32 p-tiles
NC_CH = C // 128  # 4 contraction chunks of 128
NH_CH = H // 128
F32 = mybir.dt.float32
BF16 = mybir.dt.bfloat16
FP8 = mybir.dt.float8e4
AF = mybir.ActivationFunctionType
ALU = mybir.AluOpType
PM = mybir.MatmulPerfMode


def build_nc():
    nc = bacc.Bacc("TRN2", target_bir_lowering=False, debug=False)

    # host-prepared key layouts
    # keysN[t, p, b, c]      = keys[t*128+p, b, c]            bf16 (p on partitions)
    # keysT[t, cc, b, mc, p] = keys[t*128+p, b, mc*128+cc]    fp8  (c-chunk on partitions)
    keysN_d = nc.dram_tensor("keysN", [NT, PT, BL, C], BF16, kind="ExternalInput")
    keysT_d = nc.dram_tensor(
        "keysT", [NT, 128, BL, NC_CH, PT], FP8, kind="ExternalInput"
    )
    # host-prepared weight layouts
    # waT[cc, mc, a] = Wa_w[a, mc*128+cc] fp8 ; uaT likewise bf16 ; qT[hh, hc, b] bf16
    waT_d = nc.dram_tensor("waT_h", [128, NC_CH, A], FP8, kind="ExternalInput")
    uaT_d = nc.dram_tensor("uaT_h", [128, NH_CH, A], BF16, kind="ExternalInput")
    qT_d = nc.dram_tensor("qT_h", [128, NH_CH, BL], BF16, kind="ExternalInput")
    wab_d = nc.dram_tensor("Wa_b", [A], F32, kind="ExternalInput")
    uab_d = nc.dram_tensor("Ua_b", [A], F32, kind="ExternalInput")
    vaw_d = nc.dram_tensor("va_w", [1, A], F32, kind="ExternalInput")
    vab_d = nc.dram_tensor("va_b", [1], F32, kind="ExternalInput")
    ctx_d = nc.dram_tensor("ctx_out", [BL, C], F32, kind="ExternalOutput")
    w_d = nc.dram_tensor("w_out", [P, BL], F32, kind="ExternalOutput")

    with tile.TileContext(nc) as tc:
        with (
            tc.tile_pool(name="consts", bufs=1) as consts,
            tc.tile_pool(name="setup", bufs=2) as setup,
            tc.tile_pool(name="persist", bufs=1) as persist,
            tc.tile_pool(name="keysN", bufs=4) as keysN_pool,
            tc.tile_pool(name="keysT", bufs=4) as keysT_pool,
            tc.tile_pool(name="tanh", bufs=3) as tanh_pool,
            tc.tile_pool(name="junk", bufs=2) as junk_pool,
            tc.tile_pool(name="small", bufs=4) as small_pool,
            tc.tile_pool(name="psum_a2", bufs=1, space="PSUM") as psum_a2_pool,
            tc.tile_pool(name="psum_mm1", bufs=2, space="PSUM") as psum_mm1_pool,
            tc.tile_pool(name="psum_acc", bufs=1, space="PSUM") as psum_acc_pool,
        ):
            # ---------------- constants / small weights ----------------
            waT = consts.tile([128, NC_CH, A], FP8)
            nc.scalar.dma_start(waT[:], waT_d[:])
            uaT = consts.tile([128, NH_CH, A], BF16)
            nc.scalar.dma_start(uaT[:], uaT_d[:])
            qT = consts.tile([128, NH_CH, BL], BF16)
            nc.scalar.dma_start(qT[:], qT_d[:])

            ones_col_f32 = consts.tile([128, 1], F32)
            nc.vector.memset(ones_col_f32[:], 1.0)
            # padded-ones stationary for the attn2 add: row 0 = 1, rows 1.. = 0
            a2pad = consts.tile([128, PT], BF16)
            nc.vector.memset(a2pad[:], 0.0)
            nc.vector.memset(a2pad[0:1, :], 1.0)

            # va broadcast to all partitions, repeated 4x (for the scores reduction)
            va_f32 = setup.tile([1, A], F32)
            nc.scalar.dma_start(va_f32[:], vaw_d[:])
            va_bf_row = setup.tile([1, 4, A], BF16)
            for r in range(4):
                nc.scalar.copy(va_bf_row[0:1, r, :], va_f32[:])
            va_bc4 = consts.tile([128, 4, A], BF16)
            nc.gpsimd.partition_broadcast(
                va_bc4[:].rearrange("p r a -> p (r a)"),
                va_bf_row[:].rearrange("p r a -> p (r a)"),
            )

            # va_b broadcast (bias for exp)
            vab_sb = setup.tile([1, 1], F32)
            nc.scalar.dma_start(vab_sb[:], vab_d[:])
            vab_bc = consts.tile([128, 1], F32)
            nc.gpsimd.partition_broadcast(vab_bc[:], vab_sb[:])

            # bias_sum = Wa_b + Ua_b  [1, A]
            wab_sb = setup.tile([1, A], F32, tag="bias")
            nc.scalar.dma_start(wab_sb[:], wab_d.ap())
            uab_sb = setup.tile([1, A], F32, tag="bias")
            nc.scalar.dma_start(uab_sb[:], uab_d.ap())
            bias_bf = setup.tile([1, A], BF16, tag="biasbf")
            nc.vector.tensor_add(bias_bf[:], wab_sb[:], uab_sb[:])

            # attn2[b, a] = q[b] @ Ua^T + Ua_b + Wa_b   -> [BL, A] psum
            ones_row_bf = setup.tile([1, 128], BF16, tag="onesrow")
            nc.gpsimd.memset(ones_row_bf[:], 1.0)
            a2_ps = psum_a2_pool.tile([BL, A], F32)
            for hc in range(NH_CH):
                nc.tensor.matmul(
                    a2_ps[:],
                    qT[:, hc, :],
                    uaT[:, hc, :],
                    start=(hc == 0),
                    stop=False,
                )
            nc.tensor.matmul(
                a2_ps[:], ones_row_bf[0:1, 0:BL], bias_bf[:], start=False, stop=True
            )
            a2_sb = setup.tile([BL, A], F32)
            nc.scalar.copy(a2_sb[:], a2_ps[:])
            # move the 8 rows to partition 0; a2pk row 0 = attn2[b], rows 1.. = 0
            a2_flat = setup.tile([1, BL, A], F32)
            nc.scalar.dma_start(a2_flat[0:1, :, :], a2_sb[:, :])
            a2pk = consts.tile([128, BL, A], BF16)
            nc.vector.memset(a2pk[:], 0.0)
            nc.scalar.copy(a2pk[0:1, :, :], a2_flat[:])

            # ---------------- persistent accumulators ----------------
            w_all = persist.tile([128, NT, BL], F32)
            ctx_ps = [
                psum_acc_pool.tile([128, C], F32, tag=f"ctx{i}", name=f"ctx_ps{i}")
                for i in range(2)
            ]
            z_ps = psum_acc_pool.tile([BL, 1], F32)
            for i in range(2):
                nc.vector.memset(ctx_ps[i][:], 0.0)

            # ---------------- main loop over p-tiles ----------------
            pre = {}
            for t in range(2):
                kN = keysN_pool.tile([PT, BL, C], BF16, name=f"keysN_pre{t}")
                nc.sync.dma_start(kN[:], keysN_d[t])
                kT = keysT_pool.tile([128, BL, NC_CH, PT], FP8, name=f"keysT_pre{t}")
                nc.sync.dma_start(kT[:], keysT_d[t])
                pre[t] = (kN, kT)
            for t in range(NT):
                if t in pre:
                    keysN, keysT = pre.pop(t)
                else:
                    keysN = keysN_pool.tile([PT, BL, C], BF16, name="keysN", tag="keysN_pre0")
                    nc.sync.dma_start(keysN[:], keysN_d[t])
                    keysT = keysT_pool.tile([128, BL, NC_CH, PT], FP8, name="keysT", tag="keysT_pre0")
                    nc.sync.dma_start(keysT[:], keysT_d[t])

                mm1 = [
                    psum_mm1_pool.tile([PT, 4, A], F32, tag="mm1", name="mm1ps")
                    for _ in range(2)
                ]
                for b in range(BL):
                    half, bi = divmod(b, 4)
                    out_ps = mm1[half][:, bi, :]
                    for q in range(2):
                        nc.tensor.matmul(
                            out_ps,
                            keysT[:, b, 2 * q : 2 * q + 2, :],
                            waT[:, 2 * q : 2 * q + 2, :],
                            start=(bi % 2 == 0 and q == 0),
                            stop=False,
                            perf_mode=PM.DoubleRow,
                        )
                    # attn2 add: K=128 stationary with single 1-row, rhs row0=attn2
                    nc.tensor.matmul(
                        out_ps,
                        a2pad[:],
                        a2pk[:, b, :],
                        start=False,
                        stop=(bi % 2 == 1),
                    )
                for half in range(2):
                    t_bf = tanh_pool.tile([PT, 4, A], BF16, tag="tanh")
                    nc.scalar.activation(t_bf[:], mm1[half][:], AF.Tanh)
                    prod = junk_pool.tile([PT, 4, A], BF16, tag="jnk")
                    nc.vector.tensor_mul(prod[:], t_bf[:], va_bc4[:])
                    scores_h = small_pool.tile([128, 4], F32, tag="scores")
                    nc.vector.tensor_reduce(
                        scores_h[:], prod[:], axis=mybir.AxisListType.X, op=ALU.add
                    )
                    # exp(s + va_b) for this half, keep f32 for output
                    nc.scalar.activation(
                        w_all[:, t, 4 * half : 4 * half + 4],
                        scores_h[:],
                        AF.Exp,
                        bias=vab_bc[:],
                    )
                    exp_bf = small_pool.tile([128, 4], BF16, tag="expbf")
                    nc.scalar.copy(exp_bf[:], w_all[:, t, 4 * half : 4 * half + 4])
                    # context: ctx[b] += exp_b^T @ keysN[b]  (one [1,512] psum row per b)
                    for bi in range(4):
                        b = half * 4 + bi
                        nc.tensor.matmul(
                            ctx_ps[half][32 * bi : 32 * bi + 1, :],
                            exp_bf[:, bi : bi + 1],
                            keysN[:, b, :],
                            start=(t == 0),
                            stop=(t == NT - 1),
                            tile_position=(0, 32 * bi),
                        )
                # Z[b] += sum_p exp   (f32, reads w_all directly)
                nc.tensor.matmul(
                    z_ps[:],
                    w_all[:, t, :],
                    ones_col_f32[:],
                    start=(t == 0),
                    stop=(t == NT - 1),
                )

            # ---------------- finalize ----------------
            z_sb = setup.tile([BL, 1], F32, tag="zsb")
            nc.scalar.copy(z_sb[:], z_ps[:])
            rz = setup.tile([BL, 1], F32, tag="rz")
            nc.vector.reciprocal(rz[:], z_sb[:])

            # rz broadcast for the weights normalize
            rz_row = setup.tile([1, BL], F32, tag="rzrow")
            nc.sync.dma_start(rz_row[0:1, :], rz[:, :])
            rz_bc = setup.tile([128, BL], F32, tag="rzbc")
            nc.gpsimd.partition_broadcast(rz_bc[:], rz_row[:])

            # weights: normalize + store in 4 interleaved chunks
            w_view = w_d.ap().rearrange("(t p) b -> t p b", p=PT)
            CH = NT // 4
            for g in range(4):
                for t in range(g * CH, (g + 1) * CH):
                    nc.vector.tensor_mul(
                        w_all[:, t, :], w_all[:, t, :], rz_bc[:]
                    )
                nc.sync.dma_start(
                    w_view[g * CH : (g + 1) * CH].rearrange("t p b -> p t b"),
                    w_all[:, g * CH : (g + 1) * CH, :],
                )

            # context rows: psum [32*bi] rows -> sbuf -> gather to [BL, C]
            ctx_sb = setup.tile([128, 2, C], F32, tag="ctxsb")
            for half in range(2):
                nc.scalar.copy(ctx_sb[:, half, :], ctx_ps[half][:])
            ctx_f = setup.tile([BL, C], F32, tag="ctxf")
            for half in range(2):
                nc.sync.dma_start(
                    ctx_f[4 * half : 4 * half + 4, :],
                    ctx_sb[0:128:32, half, :],
                )
            nc.vector.tensor_scalar_mul(ctx_f[:], ctx_f[:], rz[:])
            nc.sync.dma_start(ctx_d[:], ctx_f[:])

    nc.compile()
    return nc


_NC_CACHE = None


def _get_nc():
    global _NC_CACHE
    if _NC_CACHE is None:
        _NC_CACHE = build_nc()
    return _NC_CACHE


def make_in_maps(inputs):
    """Host-side prep: keys in natural (bf16) and transposed (fp8) tile layouts."""
    keys = np.asarray(inputs["keys"])
    keysN = keys.astype(ml_dtypes.bfloat16).reshape(NT, PT, B, C)
    keysT = np.ascontiguousarray(
        keys.astype(ml_dtypes.float8_e4m3)
        .reshape(NT, PT, B, NC_CH, 128)
        .transpose(0, 4, 2, 3, 1)
    )
    # waT[cc, mc, a] = Wa_w[a, mc*128+cc]
    waT = np.ascontiguousarray(
        np.asarray(inputs["Wa_w"])
        .astype(ml_dtypes.float8_e4m3)
        .reshape(A, NC_CH, 128)
        .transpose(2, 1, 0)
    )
    uaT = np.ascontiguousarray(
        np.asarray(inputs["Ua_w"])
        .astype(ml_dtypes.bfloat16)
        .reshape(A, NH_CH, 128)
        .transpose(2, 1, 0)
    )
    q = np.asarray(inputs["queries"])  # [1, B, H]
    qT_full = (
        q[0].astype(ml_dtypes.bfloat16).reshape(B, NH_CH, 128).transpose(2, 1, 0)
    )  # [hh, hc, b]
    rep = ("Wa_b", "Ua_b", "va_w", "va_b")
    in_maps = []
    for m in range(NCORES):
        sl = slice(m * BL, (m + 1) * BL)
        im = {
            "keysN": np.ascontiguousarray(keysN[:, :, sl, :]),
            "keysT": np.ascontiguousarray(keysT[:, :, sl, :, :]),
            "waT_h": waT,
            "uaT_h": uaT,
            "qT_h": np.ascontiguousarray(qT_full[:, :, sl]),
        }
        for k in rep:
            im[k] = np.asarray(inputs[k])
        in_maps.append(im)
    return in_maps


def kernel(**inputs):
    nc = _get_nc()
    in_maps = make_in_maps(inputs)
    res = bass_utils.run_bass_kernel_spmd(nc, in_maps, core_ids=list(range(NCORES)))
    ctx = np.zeros((1, B, C), np.float32)
    w = np.zeros((P, B, 1), np.float32)
    for m in range(NCORES):
        sl = slice(m * BL, (m + 1) * BL)
        ctx[0, sl, :] = res.results[m]["ctx_out"]
        w[:, sl, 0] = res.results[m]["w_out"]
    return ctx, w
